# revision 2
# baseline (speedup 1.0000x reference)
"""Trainium2 Bass kernel for nn_AttentionShareLocal — v5 (single core).

v2's exact per-window compute (4-bank S^T with row-group concurrency, proven
numerics) + two pipeline fixes:
  - software-pipelined emission: per step emit QK/exp/mul for window w and
    PV/normalize for window w-LAG, so the PE's in-order queue never waits
    inline on the ACT->DVE chain;
  - deep DMA prefetch: smaller groups (GRPQ=32 / GRPV=16) with 3-4 buffers,
    all DMAs on the otherwise-idle SP queue.
"""
import numpy as np
import ml_dtypes

import concourse.bass as bass
import concourse.tile as tile
from concourse import bacc, mybir
from concourse.bass_utils import run_bass_kernel_spmd

F32 = mybir.dt.float32
BF16 = mybir.dt.bfloat16

NCORES = 8
B, N, C = 2048, 49, 256
NH, D = 8, 32
GS = 7
WPC = B // NCORES          # windows per core = 256
GRPQ = 64                  # windows per q/k DMA group
GRPV = 32                  # windows per v/out DMA group
NG = WPC // GRPQ
NGV = WPC // GRPV
LAG = 3                    # software pipeline depth


def _build(wpc=WPC, num_devices=NCORES, repeat=1):
    ng = wpc // GRPQ
    ngv = wpc // GRPV
    nc = bacc.Bacc("TRN2", target_bir_lowering=False, debug=False,
                   num_devices=num_devices)
    qt_d = nc.declare_dram_parameter("qt", [ng * 2 * 128, GRPQ * N], BF16,
                                     isOutput=False)
    kt_d = nc.declare_dram_parameter("kt", [ng * 2 * 128, GRPQ * N], BF16,
                                     isOutput=False)
    va_d = nc.declare_dram_parameter("va", [ngv * N, GRPV * NH * 33], BF16,
                                     isOutput=False)
    expbT = nc.declare_dram_parameter("expbT", [N, NH * N], BF16,
                                      isOutput=False)
    out = nc.declare_dram_parameter("out", [ngv * N, GRPV * C], BF16,
                                    isOutput=True)

    qt_v = qt_d[:].rearrange("(g c p) x -> g c p x", c=2, p=128)
    kt_v = kt_d[:].rearrange("(g c p) x -> g c p x", c=2, p=128)
    va_v = va_d[:].rearrange("(g j) x -> g j x", j=N)
    out_v = out[:].rearrange("(g j) x -> g j x", j=N)

    with tile.TileContext(nc) as tc:
        with tc.tile_pool(name="const", bufs=1) as cpool, \
             tc.tile_pool(name="tsp", bufs=3) as tsp, \
             tc.tile_pool(name="et", bufs=4) as etp, \
             tc.tile_pool(name="io", bufs=2) as iop, \
             tc.tile_pool(name="sm", bufs=4) as smp, \
             tc.tile_pool(name="ps", bufs=1, space="PSUM") as ps, \
             tc.tile_pool(name="ps2", bufs=4, space="PSUM") as ps2:

            eb_sb = cpool.tile([N, NH * N], BF16)
            nc.sync.dma_start(eb_sb[:], expbT[:])

            qk_groups = {}
            vcur = {}
            ocur = {}

            def issue_qk(g):
                tiles = {}
                for nm, srcv in (("q", qt_v), ("k", kt_v)):
                    for cc in range(2):
                        t = tsp.tile([128, GRPQ * N], BF16,
                                     tag=f"{nm}t{cc}", name=f"{nm}{cc}")
                        nc.sync.dma_start(t[:], srcv[g, cc])
                        tiles[(nm, cc)] = t
                qk_groups[g] = tiles

            def issue_v(gv):
                vt = iop.tile([N, GRPV * NH * 33], BF16, tag="vpl",
                              name="vt")
                nc.sync.dma_start(vt[:], va_v[gv])
                vcur[gv] = vt

            def front(w):
                g, wq = divmod(w, GRPQ)
                qk = qk_groups[g]

                # QK^T into 4 PSUM banks (bank = h%4, matching the 4
                # concurrent PE row groups); chunks sequential per bank.
                sT = ps.tile([N, 4 * 512], F32, tag="sT")
                for h in range(NH):
                    ch, r = divmod(h, 4)
                    col = 512 * r + N * ch
                    nc.tensor.matmul(
                        sT[:, col:col + N],
                        qk[("k", ch)][32 * r:32 * r + 32, N * wq:N * wq + N],
                        qk[("q", ch)][32 * r:32 * r + 32, N * wq:N * wq + N],
                        start=True, stop=True,
                        tile_position=(32 * r, 0))

                sview = sT[:].rearrange("p (b c) -> p b c", b=4)[:, :, 0:2 * N]
                e0 = etp.tile([N, NH * N], BF16, tag="e0")
                nc.scalar.activation(
                    e0[:].rearrange("p (b c) -> p b c", b=4), sview,
                    mybir.ActivationFunctionType.Exp)
                eT = etp.tile([N, NH * N], BF16, tag="eT")
                nc.vector.tensor_mul(eT[:], e0[:], eb_sb[:])
                if wq == GRPQ - 1:
                    del qk_groups[g]
                return eT

            def back(w, eT):
                gv, wi = divmod(w, GRPV)
                if wi == 0:
                    ocur[gv] = iop.tile([N, GRPV * C], BF16, tag="o8",
                                        name="ot")
                o8 = ocur[gv]
                v4 = vcur[gv][:].rearrange("p (w h c) -> p w h c",
                                           w=GRPV, h=NH)
                oP = ps2.tile([N, NH * 33], F32, tag="oP")
                for h in range(NH):
                    ch, r = divmod(h, 4)
                    ecol = 2 * N * r + N * ch
                    nc.tensor.matmul(
                        oP[:, 33 * h:33 * (h + 1)],
                        eT[:, ecol:ecol + N],
                        v4[:, wi, h, :],
                        start=True, stop=True)
                ov = oP[:].rearrange("p (h c) -> p h c", h=NH)
                rt = smp.tile([N, NH], F32, tag="rt")
                nc.vector.reciprocal(rt[:], ov[:, :, 32])
                nc.vector.tensor_tensor(
                    o8[:, C * wi:C * (wi + 1)].rearrange(
                        "p (h c) -> p h c", h=NH),
                    ov[:, :, 0:32],
                    rt[:].unsqueeze(2).to_broadcast([N, NH, 32]),
                    mybir.AluOpType.mult)
                if wi == GRPV - 1:
                    nc.sync.dma_start(out_v[gv], o8[:])
                    del vcur[gv], ocur[gv]

            for _rep in range(repeat):
                pend = {}
                issue_qk(0)
                issue_v(0)
                for w in range(wpc + LAG):
                    if w < wpc:
                        nq = w + 32
                        if nq < wpc and nq % GRPQ == 0:
                            issue_qk(nq // GRPQ)
                        nv = w + 16
                        if nv < wpc and nv % GRPV == 0:
                            issue_v(nv // GRPV)
                        pend[w] = front(w)
                    if w >= LAG:
                        back(w - LAG, pend.pop(w - LAG))
    nc.compile()
    return nc


_CACHE = {}
TRACE = False
LAST_EXEC_NS = None


def _get_nc():
    if "nc" not in _CACHE:
        _CACHE["nc"] = _build()
    return _CACHE["nc"]


def _bias_table_host(W1, b1, W2, b2):
    r = np.arange(1 - GS, GS, dtype=np.float64)
    bh, bw = np.meshgrid(r, r, indexing="ij")
    biases = np.stack([bh.ravel(), bw.ravel()], axis=1)          # (169,2)
    pos = np.maximum(biases @ W1.astype(np.float64) + b1.astype(np.float64),
                     0.0) @ W2.astype(np.float64) + b2.astype(np.float64)
    coords = np.stack(np.meshgrid(np.arange(GS), np.arange(GS), indexing="ij"))
    cf = coords.reshape(2, -1)
    rel = (cf[:, :, None] - cf[:, None, :]).transpose(1, 2, 0).copy()
    rel[..., 0] += GS - 1
    rel[..., 1] += GS - 1
    rel[..., 0] *= 2 * GS - 1
    idx = rel.sum(-1)                                            # (49,49)
    return pos[idx].transpose(2, 0, 1)                           # (h,49,49)


def _prep_inputs(q, k, v, W1, b1, W2, b2):
    q = np.asarray(q, dtype=np.float32)
    k = np.asarray(k, dtype=np.float32)
    v = np.asarray(v, dtype=np.float32)

    bias = _bias_table_host(np.asarray(W1), np.asarray(b1),
                            np.asarray(W2), np.asarray(b2))      # (h,i,j)
    eb = np.exp(bias)
    # expbT[j, 98*(h%4) + 49*(h//4) + i] = exp(bias[h,i,j])  (bank-major)
    expbT = np.zeros((N, NH * N), np.float32)
    for h in range(NH):
        col = 98 * (h % 4) + 49 * (h // 4)
        expbT[:, col:col + N] = eb[h].T
    expbT = expbT.astype(ml_dtypes.bfloat16)

    scale = np.float32(D) ** np.float32(-0.5)
    qs = (q * scale).astype(ml_dtypes.bfloat16)
    kb = k.astype(ml_dtypes.bfloat16)
    qT = qs.reshape(NCORES, NG, GRPQ, N, 2, 128).transpose(0, 1, 4, 5, 2, 3)
    kT = kb.reshape(NCORES, NG, GRPQ, N, 2, 128).transpose(0, 1, 4, 5, 2, 3)
    qT = np.ascontiguousarray(qT).reshape(NCORES, NG * 2 * 128, GRPQ * N)
    kT = np.ascontiguousarray(kT).reshape(NCORES, NG * 2 * 128, GRPQ * N)
    va = np.ones((B, N, NH, 33), ml_dtypes.bfloat16)
    va[:, :, :, 0:32] = v.astype(ml_dtypes.bfloat16).reshape(B, N, NH, 32)
    va = va.reshape(NCORES, NGV, GRPV, N, NH * 33).transpose(0, 1, 3, 2, 4)
    va = np.ascontiguousarray(va).reshape(NCORES, NGV * N, GRPV * NH * 33)

    in_maps = []
    for c in range(NCORES):
        in_maps.append({
            "qt": qT[c],
            "kt": kT[c],
            "va": va[c],
            "expbT": expbT,
        })
    return in_maps


def _post(raw_outs):
    o = np.stack([np.asarray(r) for r in raw_outs])
    o = o.reshape(NCORES, NGV, N, GRPV, C).transpose(0, 1, 3, 2, 4)
    return np.ascontiguousarray(o).reshape(B, N, C).astype(np.float32)


def kernel(q, k, v, W1, b1, W2, b2, H=56, W=56):
    in_maps = _prep_inputs(q, k, v, W1, b1, W2, b2)
    nc = _get_nc()
    if TRACE:
        return _timed_run(nc, in_maps)
    res = run_bass_kernel_spmd(nc, in_maps, core_ids=list(range(NCORES)))
    return _post([res.results[c]["out"] for c in range(NCORES)])


def _timed_run(nc, in_maps, iters=50):
    import time
    import jax
    from jax.sharding import Mesh, PartitionSpec
    from jax.experimental.shard_map import shard_map
    from concourse import bass2jax as b2j
    from concourse import mybir as mb

    b2j.install_neuronx_cc_hook()
    in_names, out_names, out_avals, zero_outs = [], [], [], []
    pname = nc.partition_id_tensor.name if nc.partition_id_tensor else None
    for alloc in nc.m.functions[0].allocations:
        if not isinstance(alloc, mb.MemoryLocationSet):
            continue
        name = alloc.memorylocations[0].name
        if alloc.kind == "ExternalInput":
            if name != pname:
                in_names.append(name)
        elif alloc.kind == "ExternalOutput":
            out_names.append(name)
            shape = tuple(alloc.tensor_shape)
            dtype = mb.dt.np(alloc.dtype)
            out_avals.append(jax.core.ShapedArray(shape, dtype))
            zero_outs.append(np.zeros(shape, dtype))
    n_params = len(in_names)
    all_in_names = list(in_names) + list(out_names)
    if pname is not None:
        all_in_names.append(pname)

    def _body(*args):
        operands = list(args)
        if pname is not None:
            operands.append(b2j.partition_id_tensor())
        return tuple(b2j._bass_exec_p.bind(
            *operands,
            out_avals=tuple(out_avals),
            in_names=tuple(all_in_names),
            out_names=tuple(out_names),
            lowering_input_output_aliases=(),
            sim_require_finite=True,
            sim_require_nnan=True,
            nc=nc,
        ))

    devices = jax.devices()[:NCORES]
    mesh = Mesh(np.asarray(devices), ("core",))
    nin = n_params + len(zero_outs)
    sharded = jax.jit(shard_map(
        _body, mesh=mesh, in_specs=(PartitionSpec("core"),) * nin,
        out_specs=(PartitionSpec("core"),) * len(out_names), check_rep=False),
        keep_unused=True)

    concat_in = [np.concatenate([np.asarray(in_maps[c][nm])
                                 for c in range(NCORES)], axis=0)
                 for nm in in_names]
    concat_zeros = [np.zeros((NCORES * z.shape[0], *z.shape[1:]), z.dtype)
                    for z in zero_outs]
    dev_in = [jax.device_put(a) for a in concat_in + concat_zeros]

    out = sharded(*dev_in)
    jax.block_until_ready(out)

    @jax.jit
    def triv(x):
        return x * 2.0
    small = jax.device_put(np.zeros((NCORES * 8,), np.float32),
                           jax.sharding.NamedSharding(mesh, PartitionSpec("core")))
    jax.block_until_ready(triv(small))

    # Alternate dispatch-baseline and kernel loops over several rounds and
    # take the best-paired difference: the RPC dispatch overhead drifts with
    # ambient load, so a single round is +-1 ms noisy.
    diffs = []
    for rnd in range(5):
        o2 = small
        t0 = time.time()
        for _ in range(iters):
            o2 = triv(o2)
        jax.block_until_ready(o2)
        t_base = (time.time() - t0) / iters
        t0 = time.time()
        for _ in range(iters):
            out = sharded(*dev_in)
        jax.block_until_ready(out)
        t_kernel = (time.time() - t0) / iters
        diffs.append(t_kernel - t_base)
        print(f"round {rnd}: kernel {t_kernel*1e6:.1f} us/iter, "
              f"dispatch baseline {t_base*1e6:.1f} us/iter, "
              f"diff {(t_kernel-t_base)*1e6:.1f} us")

    global LAST_EXEC_NS
    LAST_EXEC_NS = int(max(0.0, min(diffs)) * 1e9)

    res = [np.asarray(out[0]).reshape(NCORES, *out_avals[0].shape)[c]
           for c in range(NCORES)]
    return _post(res)



# revision 22
# speedup vs baseline: 93.0194x; 93.0194x over previous
"""Trainium2 Bass kernel for nn_AttentionShareLocal — v8 (8-core SPMD,
window-pair packing, uniform 64x64 PE tiling).

Per-core shard: 256 windows processed as 128 window pairs; window A lives
at SBUF/PSUM partitions 0-48, window B at 64-112, so every ACT/DVE op
covers two windows.

All matmuls run in the 64x64 tiling mode (quadrant tiles), which needs a
zero-padded q: for the QK^T of head r (chunk ch), the stationary operand
is the raw k slice [64 rows, 49] holding heads 2rb,2rb+1 (rb = r//2), and
the moving operand is a q column block whose wrong-head rows are ZERO, so
cross-head terms vanish.  Head r sits at rows 32r in both the plain and
padded layouts, so the padded tile is built by a scatter-write DMA from
the plain DRAM q (contiguous HBM reads, strided SBUF writes — no extra
HBM traffic) into memset-once static tiles.

Uniform 64x64 mode gives: no PE tiling-mode-switch drains, 2 PSUM banks
per pair for S^T (double-buffered: no write-after-read stall between
pair t+1's QK and pair t's exp), one exp per pair.  Concurrent matmul
drains never share a (bank, partition-half): bank <- row-block rb,
partition half <- window.  PV packs the two windows into diagonal
quadrants (0,0)/(64,64); reciprocal+normalize batch over 2 pairs via
2-bank oP super-tiles.  The exp-bias multiply alternates DVE/GPSIMD.

B=2048 windows are sharded 8 ways (pure data parallel, 256 windows/core);
the bias table is replicated.
"""
import numpy as np
import ml_dtypes

import concourse.bass as bass
import concourse.tile as tile
from concourse import bacc, mybir
from concourse.bass_utils import run_bass_kernel_spmd

F32 = mybir.dt.float32
BF16 = mybir.dt.bfloat16

NCORES = 8
B, N, C = 2048, 49, 256
NH, D = 8, 32
GS = 7
WPC = B // NCORES          # windows per core = 256
NPAIR = WPC // 2           # window pairs per core = 128
GRPQ = 64                  # windows per q/k DMA group
GRPP = 16                  # pairs per v/out DMA group (32 windows)
NG = WPC // GRPQ
NGV = NPAIR // GRPP
LAG = 3                    # software pipeline depth (pairs)


def _ecol(h):
    # e-layout column of head h: bank-major (rb = (h%4)//2), then (ch, par)
    rb = (h % 4) // 2
    return 196 * rb + 49 * (2 * (h // 4) + h % 2)


def _build(wpc=WPC, num_devices=NCORES, repeat=1):
    npair = wpc // 2
    ng = wpc // GRPQ
    ngv = npair // GRPP
    nc = bacc.Bacc("TRN2", target_bir_lowering=False, debug=False,
                   num_devices=num_devices)
    qt_d = nc.declare_dram_parameter("qt", [ng * 2 * 128, GRPQ * N], BF16,
                                     isOutput=False)
    kt_d = nc.declare_dram_parameter("kt", [ng * 2 * 128, GRPQ * N], BF16,
                                     isOutput=False)
    va_d = nc.declare_dram_parameter("va", [ngv * 2 * N, GRPP * NH * 33],
                                     BF16, isOutput=False)
    expbT = nc.declare_dram_parameter("expbT", [128, NH * N], BF16,
                                      isOutput=False)
    out = nc.declare_dram_parameter("out", [ngv * 2 * N, GRPP * C], BF16,
                                    isOutput=True)

    qt_v = qt_d[:].rearrange("(g c p) x -> g c p x", c=2, p=128)
    kt_v = kt_d[:].rearrange("(g c p) x -> g c p x", c=2, p=128)
    va_v = va_d[:].rearrange("(g j) x -> g j x", j=2 * N)
    out_v = out[:].rearrange("(g j) x -> g j x", j=2 * N)

    with tile.TileContext(nc) as tc:
        with tc.tile_pool(name="const", bufs=1) as cpool, \
             tc.tile_pool(name="kp", bufs=3) as kpp, \
             tc.tile_pool(name="qz", bufs=1) as qzp, \
             tc.tile_pool(name="et", bufs=4) as etp, \
             tc.tile_pool(name="io", bufs=2) as iop, \
             tc.tile_pool(name="sm", bufs=4) as smp, \
             tc.tile_pool(name="ps", bufs=1, space="PSUM") as ps, \
             tc.tile_pool(name="ps2", bufs=1, space="PSUM") as ps2:

            eb_sb = cpool.tile([128, NH * N], BF16)
            nc.sync.dma_start(eb_sb[:], expbT[:])

            # Static zero-padded q tiles (2, rotated by group parity).
            # Column block (wq, par) holds q of head 2rb+par on rows
            # 64rb+32par and ZEROS on the complementary rows; the memset
            # writes the zeros once, the scatter DMA refills only the
            # nonzero blocks each group.
            # (the +2N column pad keeps the scatter-DMA's partition dim from
            # collapsing with the window dim into one flat stride, which
            # the byte-level race detector mis-reads as a huge span)
            qzs = []
            for _i in range(2):
                qq = [qzp.tile([128, (GRPQ + 1) * 2 * N], BF16,
                               tag=f"qz{_i}c{cc}", name=f"qz{_i}c{cc}")
                      for cc in range(2)]
                for t_ in qq:
                    nc.gpsimd.memset(t_[:], 0.0)
                qzs.append(qq)

            # Static PSUM super-tiles (2 banks each).  sS: S^T for one pair
            # (bank rb, window by partition half), double-buffered by pair
            # parity.  oP: PV output for two pairs (pair parity -> bank).
            # Matmuls only write partitions 0-48/64-112; memset once so the
            # dead partition ranges are initialized (and finite) for the
            # full-partition exp/reciprocal/normalize reads.
            sSs, oPs = [], []
            for _i in range(2):
                s0 = ps.tile([128, 1024], F32, tag=f"sS{_i}", name=f"sS{_i}")
                nc.vector.memset(s0[:], 0.0)
                sSs.append(s0)
                o0 = ps2.tile([128, 1024], F32, tag=f"oP{_i}",
                              name=f"oP{_i}")
                nc.vector.memset(o0[:], 1.0)
                oPs.append(o0)

            kt_groups = {}
            vcur = {}
            ocur = {}

            def issue_qk(g):
                tiles = {}
                for cc in range(2):
                    kt_t = kpp.tile([128, GRPQ * N], BF16,
                                    tag=f"kt{cc}", name=f"k{cc}")
                    nc.sync.dma_start(kt_t[:], kt_v[g, cc])
                    tiles[cc] = kt_t
                    qz_t = qzs[g % 2][cc]
                    for par in range(2):
                        for rb in range(2):
                            r0 = 64 * rb + 32 * par
                            src = qt_v[g, cc][r0:r0 + 32].rearrange(
                                "p (w n) -> p w n", n=N)
                            dst = qz_t[r0:r0 + 32, 0:GRPQ * 2 * N].rearrange(
                                "p (w k n) -> p w k n", k=2, n=N)[:, :, par]
                            nc.sync.dma_start(dst, src)
                kt_groups[g] = tiles

            def issue_v(gv):
                vt = iop.tile([128, GRPP * NH * 33], BF16, tag="vpl",
                              name="vt")
                nc.sync.dma_start(vt[0:N, :], va_v[gv, 0:N])
                nc.sync.dma_start(vt[64:64 + N, :], va_v[gv, N:2 * N])
                vcur[gv] = vt

            def front(t):
                g, pq = divmod(t, GRPQ // 2)
                kt = kt_groups[g]
                qz = qzs[g % 2]
                sS = sSs[t % 2]
                # QK^T, all 64x64 quadrant tiles: row block rb (heads
                # 2rb,2rb+1 of each chunk), window w -> partition half.
                # Bank rb, col 49*(2ch+par).  The two par-MMs of one
                # (rb, ch, w) share their stationary k slice.
                for rb in range(2):
                    for ch in range(2):
                        for w in range(2):
                            wq = 2 * pq + w
                            for par in range(2):
                                nc.tensor.matmul(
                                    sS[64 * w:64 * w + N,
                                       512 * rb + 49 * (2 * ch + par):
                                       512 * rb + 49 * (2 * ch + par) + N],
                                    kt[ch][64 * rb:64 * rb + 64,
                                           N * wq:N * wq + N],
                                    qz[ch][64 * rb:64 * rb + 64,
                                           2 * N * wq + N * par:
                                           2 * N * wq + N * par + N],
                                    start=True, stop=True,
                                    tile_position=(64 * rb, 64 * w))
                e0 = etp.tile([128, NH * N], BF16, tag="e0")
                sv = sS[:].rearrange("p (b x) -> p b x", b=2)[:, :, 0:196]
                nc.scalar.activation(
                    e0[:].rearrange("p (b x) -> p b x", b=2), sv,
                    mybir.ActivationFunctionType.Exp)
                eT = etp.tile([128, NH * N], BF16, tag="eT")
                # bias multiply alternates DVE / GPSIMD to split the
                # elementwise load across both engines
                if t % 2 == 0:
                    nc.vector.tensor_mul(eT[:], e0[:], eb_sb[:])
                else:
                    nc.gpsimd.tensor_tensor(eT[:], e0[:], eb_sb[:],
                                            mybir.AluOpType.mult)
                if pq == GRPQ // 2 - 1:
                    del kt_groups[g]
                return eT

            def back_mm(t, eT):
                gv, tin = divmod(t, GRPP)
                if tin == 0:
                    ocur[gv] = iop.tile([128, GRPP * C], BF16, tag="o8",
                                        name="ot")
                v4 = vcur[gv][:].rearrange("p (t h c) -> p t h c",
                                           t=GRPP, h=NH)
                u, pi = divmod(t, 2)
                oP = oPs[u % 2]
                for h in range(NH):
                    ecol = _ecol(h)
                    for w in range(2):
                        nc.tensor.matmul(
                            oP[64 * w:64 * w + N,
                               512 * pi + 33 * h:512 * pi + 33 * (h + 1)],
                            eT[64 * w:64 * w + N, ecol:ecol + N],
                            v4[64 * w:64 * w + N, tin, h, :],
                            start=True, stop=True,
                            tile_position=(64 * w, 64 * w))
                return oP

            def back_fin(u):
                # reciprocal + normalize for pairs 2u, 2u+1 in one op each
                gv, tin = divmod(2 * u, GRPP)
                o8 = ocur[gv]
                oP = oPs[u % 2]
                ov = oP[:].rearrange("p (b x) -> p b x", b=2)[:, :, 0:NH * 33]
                ov4 = ov.rearrange("p b (h c) -> p b h c", h=NH)
                rt = smp.tile([128, 2 * NH], F32, tag="rt")
                rt2 = rt[:].rearrange("p (b h) -> p b h", b=2)
                nc.vector.reciprocal(rt2, ov4[:, :, :, 32])
                nc.vector.tensor_tensor(
                    o8[:, C * tin:C * (tin + 2)].rearrange(
                        "p (b h c) -> p b h c", b=2, h=NH),
                    ov4[:, :, :, 0:32],
                    rt2.unsqueeze(3).to_broadcast([128, 2, NH, 32]),
                    mybir.AluOpType.mult)
                if tin + 1 == GRPP - 1:
                    nc.sync.dma_start(out_v[gv, 0:N], o8[0:N, :])
                    nc.sync.dma_start(out_v[gv, N:2 * N], o8[64:64 + N, :])
                    del vcur[gv], ocur[gv]

            for _rep in range(repeat):
                pend = {}
                issue_qk(0)
                issue_v(0)
                for t in range(npair + LAG):
                    if t < npair:
                        nq = t + 12
                        if nq < npair and nq % (GRPQ // 2) == 0:
                            issue_qk(nq // (GRPQ // 2))
                        nv = t + 8
                        if nv < npair and nv % GRPP == 0:
                            issue_v(nv // GRPP)
                        pend[t] = front(t)
                    if t >= LAG:
                        tb = t - LAG
                        back_mm(tb, pend.pop(tb))
                        if tb % 2 == 1:
                            back_fin(tb // 2)
    nc.compile()
    return nc


_CACHE = {}


def _get_nc():
    if "nc" not in _CACHE:
        _CACHE["nc"] = _build()
    return _CACHE["nc"]


def _bias_table_host(W1, b1, W2, b2):
    r = np.arange(1 - GS, GS, dtype=np.float64)
    bh, bw = np.meshgrid(r, r, indexing="ij")
    biases = np.stack([bh.ravel(), bw.ravel()], axis=1)          # (169,2)
    pos = np.maximum(biases @ W1.astype(np.float64) + b1.astype(np.float64),
                     0.0) @ W2.astype(np.float64) + b2.astype(np.float64)
    coords = np.stack(np.meshgrid(np.arange(GS), np.arange(GS), indexing="ij"))
    cf = coords.reshape(2, -1)
    rel = (cf[:, :, None] - cf[:, None, :]).transpose(1, 2, 0).copy()
    rel[..., 0] += GS - 1
    rel[..., 1] += GS - 1
    rel[..., 0] *= 2 * GS - 1
    idx = rel.sum(-1)                                            # (49,49)
    return pos[idx].transpose(2, 0, 1)                           # (h,49,49)


def _prep_inputs(q, k, v, W1, b1, W2, b2):
    q = np.asarray(q, dtype=np.float32)
    k = np.asarray(k, dtype=np.float32)
    v = np.asarray(v, dtype=np.float32)

    bias = _bias_table_host(np.asarray(W1), np.asarray(b1),
                            np.asarray(W2), np.asarray(b2))      # (h,i,j)
    eb = np.exp(bias)
    # expbT[64*w + j, _ecol(h) + i] = exp(bias[h,i,j]) for both window
    # partition slots
    expbT = np.zeros((128, NH * N), np.float32)
    for h in range(NH):
        col = _ecol(h)
        expbT[0:N, col:col + N] = eb[h].T
        expbT[64:64 + N, col:col + N] = eb[h].T
    expbT = expbT.astype(ml_dtypes.bfloat16)

    scale = np.float32(D) ** np.float32(-0.5)
    qs = (q * scale).astype(ml_dtypes.bfloat16)
    kb = k.astype(ml_dtypes.bfloat16)
    qT = qs.reshape(NCORES, NG, GRPQ, N, 2, 128).transpose(0, 1, 4, 5, 2, 3)
    kT = kb.reshape(NCORES, NG, GRPQ, N, 2, 128).transpose(0, 1, 4, 5, 2, 3)
    qT = np.ascontiguousarray(qT).reshape(NCORES, NG * 2 * 128, GRPQ * N)
    kT = np.ascontiguousarray(kT).reshape(NCORES, NG * 2 * 128, GRPQ * N)

    va = np.ones((B, N, NH, 33), ml_dtypes.bfloat16)
    va[:, :, :, 0:32] = v.astype(ml_dtypes.bfloat16).reshape(B, N, NH, 32)
    # rows: gv*98 + (0-48: even window, 49-97: odd window), cols: pair-major
    va = va.reshape(NCORES, NGV, GRPP, 2, N, NH * 33)
    va = va.transpose(0, 1, 3, 4, 2, 5)
    va = np.ascontiguousarray(va).reshape(NCORES, NGV * 2 * N,
                                          GRPP * NH * 33)

    in_maps = []
    for c in range(NCORES):
        in_maps.append({
            "qt": qT[c],
            "kt": kT[c],
            "va": va[c],
            "expbT": expbT,
        })
    return in_maps


def _post(raw_outs):
    o = np.stack([np.asarray(r) for r in raw_outs])
    o = o.reshape(NCORES, NGV, 2, N, GRPP, C).transpose(0, 1, 4, 2, 3, 5)
    return np.ascontiguousarray(o).reshape(B, N, C).astype(np.float32)


def kernel(q, k, v, W1, b1, W2, b2, H=56, W=56):
    in_maps = _prep_inputs(q, k, v, W1, b1, W2, b2)
    nc = _get_nc()
    res = run_bass_kernel_spmd(nc, in_maps, core_ids=list(range(NCORES)))
    return _post([res.results[c]["out"] for c in range(NCORES)])
